# revision 1
# baseline (speedup 1.0000x reference)
"""Trainium2 Bass kernel for LocalSquaredDistanceLayer (shapelet min-distance).

Math (matching the reference exactly):
  x_norm   = z-normalize x over time per (batch, channel)
  kern     = z-normalize kernel per shapelet over (KSZ, C)
  For output element out[b, t, k'] with k' = 4*ch + j (ch = k'//4, j = k'%4):
     w = x_norm[b, t+8j : t+8j+8, ch]               (8 consecutive samples)
     out[b,t,k'] = min_s || w - kern[s, k', :] ||^2
  (This is the tf.extract_patches transpose/reshape identity: the patch
   vector for k' is 8 consecutive time samples of channel k'//4 offset 8*(k'%4).)

Device algorithm per core (2 batches per core, kernel replicated):
  - transpose x to per-(b,ch) time signals, z-normalize, square
  - build Hankel tiles H[sig] (65, 512): rows 0-31 x-shifts, 32-63 x^2 shifts,
    row 64 ones
  - build filter tiles F[ch] (65, 256): block-diagonal taps (-2*kern^T),
    ones blocks (for the x^2 window sum), K2 row (||kern||^2) so that one
    matmul psum[t, 4ch+j*? ...] = full squared distance
  - matmul per (b, tchunk, ch): psum (128, 8*256) = distances for 32 k'
    groups x 64 shapelets
  - min-reduce over the innermost 64 (shapelets), DMA out
"""

import sys

for _p in ("/opt/trn_rl_repo",):
    if _p not in sys.path:
        sys.path.insert(0, _p)

import numpy as np

B, T, C = 16, 512, 8
S, KSZ = 64, 32
TOUT = T - KSZ + 1  # 481
NCORES = 8
BPC = B // NCORES  # batches per core
NSIG = BPC * C  # signals per core
EPS = 1e-8
XPAD = 544  # padded signal length (hankel reads up to 511+31)

_cache = {}


def _rap(base, dims):
    """Raw AP at base slice's offset with explicit [step, count] dims (flat elems)."""
    import concourse.bass as bass

    return bass.AP(tensor=base.tensor, offset=base.offset, ap=[list(d) for d in dims])


def _build_nc():
    import concourse.bass as bass
    import concourse.bacc as bacc
    import concourse.tile as tile
    from concourse import mybir
    from concourse.masks import make_identity
    from contextlib import ExitStack

    f32 = mybir.dt.float32
    AX = mybir.AxisListType
    OP = mybir.AluOpType
    ACT = mybir.ActivationFunctionType

    nc = bacc.Bacc("TRN2", target_bir_lowering=False, debug=False)
    x_d = nc.dram_tensor("x", [BPC, T, C], f32, kind="ExternalInput").ap()
    k_d = nc.dram_tensor("kernel", [S, KSZ, C], f32, kind="ExternalInput").ap()
    o_d = nc.dram_tensor("out", [BPC, TOUT, KSZ], f32, kind="ExternalOutput").ap()

    with tile.TileContext(nc) as tc, ExitStack() as ctx:
        const = ctx.enter_context(tc.tile_pool(name="const", bufs=1))
        outp = ctx.enter_context(tc.tile_pool(name="outp", bufs=4))
        dram = ctx.enter_context(tc.tile_pool(name="dram", bufs=1, space="DRAM"))

        ident = const.tile([128, 128], f32, tag="ident")
        make_identity(nc, ident[:])

        F_tiles = [const.tile([65, 256], f32, tag=f"F{ch}", name=f"F{ch}") for ch in range(C)]
        ones8 = const.tile([8, 64], f32, tag="ones8")
        nc.vector.memset(ones8[:], 1.0)
        onesD = dram.tile([8, 64], f32, tag="onesD")
        nc.sync.dma_start(out=onesD[:], in_=ones8[:])
        H_tiles = [const.tile([65, 512], f32, tag=f"H{s}", name=f"H{s}") for s in range(NSIG)]
        Xn = const.tile([NSIG, XPAD], f32, tag="Xn")
        X2n = const.tile([NSIG, XPAD], f32, tag="X2n")

        with tc.tile_pool(name="pprep", bufs=1, space="PSUM") as pprep, \
             tc.tile_pool(name="ldp", bufs=2) as ldp:
            # ---- kernel prep ----
            KN = const.tile([S, KSZ * C], f32, tag="KN")
            nc.sync.dma_start(out=KN[:], in_=k_d.rearrange("s k c -> s (k c)"))
            kst = ldp.tile([S, nc.vector.BN_STATS_DIM], f32, tag="kst")
            nc.vector.bn_stats(out=kst[:], in_=KN[:])
            mvk = ldp.tile([S, nc.vector.BN_AGGR_DIM], f32, tag="mvk")
            nc.vector.bn_aggr(out=mvk[:], in_=kst[:])
            kstd = ldp.tile([S, 1], f32, tag="kstd")
            nc.scalar.activation(out=kstd[:], in_=mvk[:, 1:2], func=ACT.Sqrt)
            nc.vector.tensor_scalar_add(kstd[:], kstd[:], EPS)
            krstd = ldp.tile([S, 1], f32, tag="krstd")
            nc.vector.reciprocal(out=krstd[:], in_=kstd[:])
            # KNm = -2 * (KN - mean) * rstd  => scale=-2*rstd, bias=2*mean*rstd
            kscale = ldp.tile([S, 1], f32, tag="kscale")
            nc.vector.tensor_scalar_mul(kscale[:], krstd[:], -2.0)
            kbias = ldp.tile([S, 1], f32, tag="kbias")
            nc.vector.scalar_tensor_tensor(
                out=kbias[:], in0=mvk[:, 0:1], scalar=2.0, in1=krstd[:],
                op0=OP.mult, op1=OP.mult)
            KNm = const.tile([S, KSZ * C], f32, tag="KNm")
            nc.vector.tensor_scalar(
                out=KNm[:], in0=KN[:], scalar1=kscale[:], scalar2=kbias[:],
                op0=OP.mult, op1=OP.add)
            # K2[s,k'] = sum_c kern_n^2 = 0.25 * sum_c KNm^2
            KN2 = ldp.tile([S, KSZ * C], f32, tag="KN2")
            nc.scalar.activation(out=KN2[:], in_=KNm[:], func=ACT.Square)
            K2sn = const.tile([S, KSZ], f32, tag="K2sn")
            nc.vector.tensor_reduce(
                out=K2sn[:], in_=KN2[:].rearrange("s (k c) -> s k c", c=C),
                axis=AX.X, op=OP.add)
            nc.vector.tensor_scalar_mul(K2sn[:], K2sn[:], 0.25)

            # transpose KNm (64, 256) -> TP (8, 32*64) psum, slice per k'
            TP = pprep.tile([8, KSZ * S], f32, tag="TP")
            for kp in range(KSZ):
                nc.tensor.transpose(
                    TP[:, kp * S:(kp + 1) * S], KNm[:, kp * C:(kp + 1) * C],
                    ident[0:S, 0:S])
            K2T = pprep.tile([KSZ, S], f32, tag="K2T")
            nc.tensor.transpose(K2T[:], K2sn[:], ident[0:S, 0:S])
            # psum is not DMA-addressable: stage to SBUF via ACT copies
            Fx = ldp.tile([8, KSZ * S], f32, tag="Fx")
            nc.scalar.copy(out=Fx[:], in_=TP[:])
            K2sb = ldp.tile([KSZ, S], f32, tag="K2sb")
            nc.scalar.copy(out=K2sb[:], in_=K2T[:])
            FxD = dram.tile([8, KSZ * S], f32, tag="FxD")
            nc.sync.dma_start(out=FxD[:], in_=Fx[:])
            K2D = dram.tile([KSZ, S], f32, tag="K2D")
            nc.sync.dma_start(out=K2D[:], in_=K2sb[:])

            # ---- F tile zero-fill (scatter happens after the barrier) ----
            for ch in range(C):
                nc.vector.memset(F_tiles[ch][:], 0.0)

            # ---- x load + transpose to signals ----
            PX = pprep.tile([8, BPC * T], f32, tag="PX")
            for b in range(BPC):
                for cc in range(4):
                    X0 = ldp.tile([128, C], f32, tag="X0")
                    nc.sync.dma_start(out=X0[:], in_=x_d[b, cc * 128:(cc + 1) * 128, :])
                    nc.tensor.transpose(
                        PX[:, b * T + cc * 128: b * T + (cc + 1) * 128], X0[:],
                        ident[:, :])
            Xst = ldp.tile([8, BPC * T], f32, tag="Xst")
            nc.scalar.copy(out=Xst[:], in_=PX[:])
            Xsig = ldp.tile([NSIG, T], f32, tag="Xsig")
            for b in range(BPC):
                nc.sync.dma_start(
                    out=Xsig[b * C:(b + 1) * C, :], in_=Xst[:, b * T:(b + 1) * T])

            # ---- x normalize ----
            xst = ldp.tile([NSIG, nc.vector.BN_STATS_DIM], f32, tag="xst")
            nc.vector.bn_stats(out=xst[:], in_=Xsig[:])
            mvx = ldp.tile([NSIG, nc.vector.BN_AGGR_DIM], f32, tag="mvx")
            nc.vector.bn_aggr(out=mvx[:], in_=xst[:])
            xstd = ldp.tile([NSIG, 1], f32, tag="xstd")
            nc.scalar.activation(out=xstd[:], in_=mvx[:, 1:2], func=ACT.Sqrt)
            nc.vector.tensor_scalar_add(xstd[:], xstd[:], EPS)
            xrstd = ldp.tile([NSIG, 1], f32, tag="xrstd")
            nc.vector.reciprocal(out=xrstd[:], in_=xstd[:])
            xbias = ldp.tile([NSIG, 1], f32, tag="xbias")
            nc.vector.scalar_tensor_tensor(
                out=xbias[:], in0=mvx[:, 0:1], scalar=-1.0, in1=xrstd[:],
                op0=OP.mult, op1=OP.mult)
            nc.vector.memset(Xn[:], 0.0)
            nc.vector.memset(X2n[:], 0.0)
            nc.vector.tensor_scalar(
                out=Xn[:, 0:T], in0=Xsig[:], scalar1=xrstd[:], scalar2=xbias[:],
                op0=OP.mult, op1=OP.add)
            nc.scalar.activation(out=X2n[:, 0:T], in_=Xn[:, 0:T], func=ACT.Square)

            # ---- stage normalized signals to DRAM; H ones rows ----
            XnD = dram.tile([NSIG, XPAD], f32, tag="XnD")
            nc.sync.dma_start(out=XnD[:], in_=Xn[:])
            X2nD = dram.tile([NSIG, XPAD], f32, tag="X2nD")
            nc.sync.dma_start(out=X2nD[:], in_=X2n[:])
            for sig in range(NSIG):
                nc.vector.memset(H_tiles[sig][64:65, :], 1.0)

            # ---- single sync point: all staging/memsets above, all
            # scatter DMAs below (keeps per-DMA wait counts at 1) ----
            tc.strict_bb_all_engine_barrier()

            # ---- F tile scatter ----
            for ch in range(C):
                Fc = F_tiles[ch]
                for j in range(4):
                    kp = 4 * ch + j
                    # tap block: F[8j+c, 64j+s] = -2*kern_n[s, 4ch+j, c]
                    nc.sync.dma_start(
                        out=Fc[8 * j:8 * j + 8, S * j:S * (j + 1)],
                        in_=FxD[:, kp * S:(kp + 1) * S])
                    # ones block for x^2 rows
                    nc.sync.dma_start(
                        out=Fc[32 + 8 * j:40 + 8 * j, S * j:S * (j + 1)],
                        in_=onesD[:])
                    # K2 row segment
                    nc.sync.dma_start(
                        out=Fc[64:65, S * j:S * (j + 1)],
                        in_=K2D[kp:kp + 1, :])

            # ---- H tiles (hankels via DRAM shifted reads) ----
            for sig in range(NSIG):
                Hs = H_tiles[sig]
                nc.sync.dma_start(
                    out=Hs[0:KSZ, :],
                    in_=_rap(XnD[sig:sig + 1, 0:1], [[1, KSZ], [1, T]]))
                nc.sync.dma_start(
                    out=Hs[KSZ:2 * KSZ, :],
                    in_=_rap(X2nD[sig:sig + 1, 0:1], [[1, KSZ], [1, T]]))

            # funnel all scatter-DMA completions through one sync point so
            # the matmuls each carry a single wait
            tc.strict_bb_all_engine_barrier()

        # ---- main: matmuls + min-reduce + store ----
        with tc.tile_pool(name="pmm", bufs=2, space="PSUM") as pmm:
            for b in range(BPC):
                for cc in range(4):
                    c0 = cc * 128
                    cnt = 128 if cc < 3 else TOUT - 3 * 128
                    acc = pmm.tile([128, C * 256], f32, tag="acc")
                    for ch in range(C):
                        nc.tensor.matmul(
                            acc[:, ch * 256:(ch + 1) * 256],
                            lhsT=H_tiles[b * C + ch][:, c0:c0 + 128],
                            rhs=F_tiles[ch][:],
                            start=True, stop=True)
                    PM = outp.tile([128, KSZ], f32, tag="PM")
                    nc.vector.tensor_reduce(
                        out=PM[:],
                        in_=acc[:].rearrange("p (g s) -> p g s", s=S),
                        axis=AX.X, op=OP.min)
                    nc.sync.dma_start(
                        out=o_d[b, c0:c0 + cnt, :], in_=PM[0:cnt, :])

    nc.compile()
    return nc


def get_nc():
    if "nc" not in _cache:
        _cache["nc"] = _build_nc()
    return _cache["nc"]


def kernel(x: np.ndarray, kernel: np.ndarray) -> np.ndarray:
    from concourse.bass_utils import run_bass_kernel_spmd

    nc = get_nc()
    x = np.ascontiguousarray(x, dtype=np.float32)
    kern = np.ascontiguousarray(kernel, dtype=np.float32)
    in_maps = [
        {"x": x[i * BPC:(i + 1) * BPC], "kernel": kern} for i in range(NCORES)
    ]
    res = run_bass_kernel_spmd(nc, in_maps, core_ids=list(range(NCORES)))
    return np.concatenate([r["out"] for r in res.results], axis=0)


if __name__ == "__main__":
    rng = np.random.default_rng(0)
    x = rng.standard_normal((B, T, C), dtype=np.float32)
    k = rng.uniform(-0.05, 0.05, (S, KSZ, C)).astype(np.float32)
    out = kernel(x=x, kernel=k)
    print(out.shape, out.dtype)



# revision 17
# speedup vs baseline: 2.3104x; 2.3104x over previous
"""Trainium2 Bass kernel for LocalSquaredDistanceLayer (shapelet min-distance).

Math (matching the reference exactly):
  x_norm   = z-normalize x over time per (batch, channel)
  kern     = z-normalize kernel per shapelet over (KSZ, C)
  For output element out[b, t, kp] with kp = 4*ch + jo (ch = kp//4, jo = kp%4):
     w = x_norm[b, t+8jo : t+8jo+8, ch]               (8 consecutive samples)
     out[b,t,kp] = min_s || w - kern[s, kp, :] ||^2

Device algorithm per core (2 batches per core, kernel replicated):
  - Xsig[(b,ch), t] via gather DMA; z-normalize; stage to DRAM
  - Hall[65, 16*512]: rows 0-31 shifted x (Hankel via flat DRAM reads),
    rows 32-63 squares (computed in SBUF), row 64 ones
  - Fall[65, 8*256]: block-diagonal taps (-2*kern^T), ones blocks, K2 row,
    scattered via flat-addressed gather DMAs from DRAM-staged KNm/K2
  - per (b, tchunk): 8 fp32r matmuls psum[128, 2048] -> full squared
    distances for 32 kp-groups x 64 shapelets
  - min over shapelets split Vector/GpSimd, DMA out
"""

import sys

for _p in ("/opt/trn_rl_repo",):
    if _p not in sys.path:
        sys.path.insert(0, _p)

import numpy as np

B, T, C = 16, 512, 8
S, KSZ = 64, 32
TOUT = T - KSZ + 1  # 481
NCORES = 8
BPC = B // NCORES  # batches per core
NSIG = BPC * C  # signals per core
EPS = 1e-8
XPAD = 544  # padded signal length (hankel reads up to 511+31)

_cache = {}


def _rap(base, dims):
    """Raw AP at base slice's offset with explicit [step, count] dims."""
    import concourse.bass as bass

    return bass.AP(tensor=base.tensor, offset=base.offset, ap=[list(d) for d in dims])


def _build_nc():
    import concourse.bass as bass
    import concourse.bacc as bacc
    import concourse.tile as tile
    from concourse import mybir
    from contextlib import ExitStack

    f32 = mybir.dt.float32
    f32r = mybir.dt.float32r
    bf16 = mybir.dt.bfloat16
    AX = mybir.AxisListType
    OP = mybir.AluOpType
    ACT = mybir.ActivationFunctionType

    nc = bacc.Bacc("TRN2", target_bir_lowering=False, debug=False)
    x_d = nc.dram_tensor("x", [BPC, T, C], f32, kind="ExternalInput").ap()
    k_d = nc.dram_tensor("kernel", [S, KSZ, C], f32, kind="ExternalInput").ap()
    o_d = nc.dram_tensor("out", [BPC, TOUT, KSZ], f32, kind="ExternalOutput").ap()

    with tile.TileContext(nc) as tc, ExitStack() as ctx:
        const = ctx.enter_context(tc.tile_pool(name="const", bufs=1))
        outp = ctx.enter_context(tc.tile_pool(name="outp", bufs=4))
        dram = ctx.enter_context(tc.tile_pool(name="dram", bufs=1, space="DRAM"))

        Hall = const.tile([65, NSIG * T], f32, tag="Hall")
        Fall = const.tile([65, C * 4 * S], f32, tag="Fall")
        ones8k = const.tile([16, 512], f32, tag="ones8k")
        Xsig = const.tile([NSIG, T], f32, tag="Xsig")
        Xn = const.tile([NSIG, XPAD], f32, tag="Xn")

        with tc.tile_pool(name="ldp", bufs=1) as ldp:
            # ---- zero/one fills (no deps; overlap everything) ----
            nc.gpsimd.memset(Fall[0:64, :], 0.0)
            nc.gpsimd.memset(ones8k[:], 1.0)
            nc.vector.memset(Xn[:], 0.0)

            # ---- kernel prep chain ----
            KN = ldp.tile([S, KSZ * C], f32, tag="KN")
            nc.gpsimd.dma_start(out=KN[:], in_=k_d.rearrange("s k c -> s (k c)"))
            kst = ldp.tile([S, nc.vector.BN_STATS_DIM], f32, tag="kst")
            nc.vector.bn_stats(out=kst[:], in_=KN[:])
            mvk = ldp.tile([S, nc.vector.BN_AGGR_DIM], f32, tag="mvk")
            nc.vector.bn_aggr(out=mvk[:], in_=kst[:])
            kstd = ldp.tile([S, 1], f32, tag="kstd")
            nc.scalar.activation(out=kstd[:], in_=mvk[:, 1:2], func=ACT.Sqrt)
            nc.vector.tensor_scalar_add(kstd[:], kstd[:], EPS)
            krstd = ldp.tile([S, 1], f32, tag="krstd")
            nc.vector.reciprocal(out=krstd[:], in_=kstd[:])
            kscale = ldp.tile([S, 1], f32, tag="kscale")
            nc.vector.tensor_scalar_mul(kscale[:], krstd[:], -2.0)
            kbias = ldp.tile([S, 1], f32, tag="kbias")
            nc.vector.scalar_tensor_tensor(
                out=kbias[:], in0=mvk[:, 0:1], scalar=2.0, in1=krstd[:],
                op0=OP.mult, op1=OP.mult)
            # KNm = -2 * (KN - mean) * rstd
            KNm = ldp.tile([S, KSZ * C], f32, tag="KNm")
            nc.vector.tensor_scalar(
                out=KNm[:], in0=KN[:], scalar1=kscale[:], scalar2=kbias[:],
                op0=OP.mult, op1=OP.add)
            # K2[kp, s] = sum_c kern_n^2 = 0.25 * sum_c KNm^2
            KN2 = ldp.tile([S, KSZ * C], f32, tag="KN2")
            nc.scalar.activation(out=KN2[:], in_=KNm[:], func=ACT.Square)
            # K2sn columns in (j, ch) order so the F scatter reads
            # consecutive partitions after the transpose
            K2sn = ldp.tile([S, KSZ], f32, tag="K2sn")
            nc.vector.tensor_reduce(
                out=K2sn[:],
                in_=KN2[:].rearrange("s (ch j c) -> s j ch c", ch=C, j=4, c=C),
                axis=AX.X, op=OP.add)
            nc.vector.tensor_scalar_mul(K2sn[:], K2sn[:], 0.25)

            # ---- F staging via PE transposes ----
            # Fall free layout: col = jo*512 + ch*64 + s  (jo outermost)
            with tc.tile_pool(name="pprep", bufs=1, space="PSUM") as pprep:
                identS = ldp.tile([128, 128], f32, tag="ident")
                from concourse.masks import make_identity
                make_identity(nc, identS[:])
                # taps transposed: TP[c, jo*512 + ch*64 + s] = KNm[s, (4ch+jo)*8+c]
                TP = pprep.tile([C, 4 * C * S], f32, tag="TP")
                for kp in range(KSZ):
                    ch, jo = kp // 4, kp % 4
                    nc.tensor.transpose(
                        TP[:, jo * 512 + ch * S:jo * 512 + ch * S + S],
                        KNm[:, kp * C:(kp + 1) * C],
                        identS[0:S, 0:S])
                K2T = pprep.tile([KSZ, S], f32, tag="K2T")
                nc.tensor.transpose(K2T[:], K2sn[:], identS[0:S, 0:S])
                Fx = ldp.tile([C, 4 * C * S], f32, tag="Fx")
                nc.scalar.copy(out=Fx[:].bitcast(f32r), in_=TP[:])
                K2sb = ldp.tile([KSZ, S], f32, tag="K2sb")
                nc.scalar.copy(out=K2sb[:].bitcast(f32r), in_=K2T[:])

            # ---- F scatter: all-2D SBUF->SBUF block DMAs ----
            for j in range(4):
                # taps: rows 8j..8j+8, cols j*512..(j+1)*512
                nc.gpsimd.dma_start(
                    out=Fall[8 * j:8 * j + 8, 512 * j:512 * (j + 1)].bitcast(f32r),
                    in_=Fx[:, 512 * j:512 * (j + 1)].bitcast(f32r))
                # ones blocks for the x^2 window sum
                nc.scalar.dma_start(
                    out=Fall[32 + 8 * j:40 + 8 * j, 512 * j:512 * (j + 1)].bitcast(f32r),
                    in_=ones8k[0:C, :].bitcast(f32r))
                # K2 row segment: Fall[64, j*512 + ch*64 + s] = K2sb[j*8+ch, s]
                nc.gpsimd.dma_start(
                    out=Fall[64:65, 512 * j:512 * (j + 1)].bitcast(f32r),
                    in_=K2sb[C * j:C * (j + 1), :].bitcast(f32r))
            # H ones row
            nc.scalar.dma_start(out=Hall[64:65, :].bitcast(f32r), in_=ones8k[:].bitcast(f32r))

            # ---- x gather to per-(b,ch) time signals ----
            for b in range(BPC):
                nc.sync.dma_start(
                    out=Xsig[b * C:(b + 1) * C, :],
                    in_=_rap(x_d[b, 0:1, 0:1], [[1, C], [C, T]]))

            # ---- x normalize ----
            xst = ldp.tile([NSIG, nc.vector.BN_STATS_DIM], f32, tag="xst")
            nc.vector.bn_stats(out=xst[:], in_=Xsig[:])
            mvx = ldp.tile([NSIG, nc.vector.BN_AGGR_DIM], f32, tag="mvx")
            nc.vector.bn_aggr(out=mvx[:], in_=xst[:])
            xstd = ldp.tile([NSIG, 1], f32, tag="xstd")
            nc.scalar.activation(out=xstd[:], in_=mvx[:, 1:2], func=ACT.Sqrt)
            nc.vector.tensor_scalar_add(xstd[:], xstd[:], EPS)
            xrstd = ldp.tile([NSIG, 1], f32, tag="xrstd")
            nc.vector.reciprocal(out=xrstd[:], in_=xstd[:])
            xbias = ldp.tile([NSIG, 1], f32, tag="xbias")
            nc.vector.scalar_tensor_tensor(
                out=xbias[:], in0=mvx[:, 0:1], scalar=-1.0, in1=xrstd[:],
                op0=OP.mult, op1=OP.mult)
            nc.vector.tensor_scalar(
                out=Xn[:, 0:T].bitcast(f32r), in0=Xsig[:], scalar1=xrstd[:],
                scalar2=xbias[:], op0=OP.mult, op1=OP.add)
            X2n = const.tile([NSIG, XPAD], f32, tag="X2n")
            nc.scalar.activation(out=X2n[:].bitcast(f32r), in_=Xn[:],
                                 func=ACT.Square)
            XnD = dram.tile([NSIG, XPAD], f32, tag="XnD")
            nc.sync.dma_start(out=XnD[:].bitcast(f32r), in_=Xn[:].bitcast(f32r))
            X2nD = dram.tile([NSIG, XPAD], f32, tag="X2nD")
            nc.scalar.dma_start(out=X2nD[:].bitcast(f32r), in_=X2n[:].bitcast(f32r))

            # ---- Hankel x rows via flat DRAM reads; squares in SBUF ----
            dma_engines = [nc.sync, nc.scalar, nc.gpsimd]
            for sig in range(NSIG):
                eng = dma_engines[sig % 3]
                eng.dma_start(
                    out=Hall[0:KSZ, sig * T:(sig + 1) * T].bitcast(f32r),
                    in_=_rap(XnD[sig:sig + 1, 0:1],
                             [[1, KSZ], [1, T]]).bitcast(f32r))
                eng2 = dma_engines[(sig + 1) % 3]
                eng2.dma_start(
                    out=Hall[KSZ:2 * KSZ, sig * T:(sig + 1) * T].bitcast(f32r),
                    in_=_rap(X2nD[sig:sig + 1, 0:1],
                             [[1, KSZ], [1, T]]).bitcast(f32r))

        # ---- main: fp32r matmuls + split min-reduce + store ----
        with tc.tile_pool(name="pmm", bufs=2, space="PSUM") as pmm, \
             tc.tile_pool(name="redp", bufs=2) as redp:
            for b in range(BPC):
                for cc in range(4):
                    c0 = cc * 128
                    cnt = 128 if cc < 3 else TOUT - 3 * 128
                    acc = pmm.tile([128, C * 4 * S], f32, tag="acc")
                    for ch in range(C):
                        sig = b * C + ch
                        # rhs strided over Fall's (jo, ch, s) layout: this
                        # ch's 4 jo-blocks of 64 shapelet columns
                        rhs = _rap(Fall[0:65, ch * S:ch * S + 1],
                                   [[2048, 65], [512, 4], [1, S]])
                        nc.tensor.matmul(
                            acc[:, ch * 256:(ch + 1) * 256],
                            lhsT=Hall[:, sig * T + c0:sig * T + c0 + 128].bitcast(f32r),
                            rhs=rhs.bitcast(f32r),
                            start=True, stop=True)
                    PM = outp.tile([128, KSZ], f32, tag="PM")
                    # scalar: copy-cast upper half of PSUM to bf16 in SBUF
                    tb = redp.tile([128, 1024], bf16, tag="tb")
                    nc.scalar.copy(out=tb[:], in_=acc[:, 1024:2048])
                    # vector: reduce lower half straight off PSUM (fp32)
                    nc.vector.tensor_reduce(
                        out=PM[:, 0:16],
                        in_=acc[:, 0:1024].rearrange("p (g s) -> p g s", s=S),
                        axis=AX.X, op=OP.min)
                    # vector: reduce upper half from SBUF bf16 at 2 elem/cyc
                    nc.vector.tensor_reduce(
                        out=PM[:, 16:32],
                        in_=tb[:].rearrange("p (g s) -> p g s", s=S),
                        axis=AX.X, op=OP.min)
                    nc.sync.dma_start(
                        out=o_d[b, c0:c0 + cnt, :], in_=PM[0:cnt, :])

    nc.compile()
    return nc


def get_nc():
    if "nc" not in _cache:
        _cache["nc"] = _build_nc()
    return _cache["nc"]


def kernel(x: np.ndarray, kernel: np.ndarray) -> np.ndarray:
    from concourse.bass_utils import run_bass_kernel_spmd

    nc = get_nc()
    x = np.ascontiguousarray(x, dtype=np.float32)
    kern = np.ascontiguousarray(kernel, dtype=np.float32)
    in_maps = [
        {"x": x[i * BPC:(i + 1) * BPC], "kernel": kern} for i in range(NCORES)
    ]
    res = run_bass_kernel_spmd(nc, in_maps, core_ids=list(range(NCORES)))
    return np.concatenate([r["out"] for r in res.results], axis=0)


if __name__ == "__main__":
    rng = np.random.default_rng(0)
    x = rng.standard_normal((B, T, C), dtype=np.float32)
    k = rng.uniform(-0.05, 0.05, (S, KSZ, C)).astype(np.float32)
    out = kernel(x=x, kernel=k)
    print(out.shape, out.dtype)


# revision 21
# speedup vs baseline: 3.0852x; 1.3353x over previous
"""Trainium2 Bass kernel for LocalSquaredDistanceLayer (shapelet min-distance).

Math (matching the reference exactly):
  x_norm   = z-normalize x over time per (batch, channel)
  kern     = z-normalize kernel per shapelet over (KSZ, C)
  For output element out[b, t, kp] with kp = 4*ch + jo (ch = kp//4, jo = kp%4):
     w = x_norm[b, t+8jo : t+8jo+8, ch]               (8 consecutive samples)
     out[b,t,kp] = min_s || w - kern[s, kp, :] ||^2

Device algorithm per core (2 batches per core, kernel replicated):
  - Xsig[(b,ch), t] via gather DMA; z-normalize; stage to DRAM
  - Hall[65, 16*512]: rows 0-31 shifted x (Hankel via flat DRAM reads),
    rows 32-63 squares (computed in SBUF), row 64 ones
  - Fall[65, 8*256]: block-diagonal taps (-2*kern^T), ones blocks, K2 row,
    scattered via flat-addressed gather DMAs from DRAM-staged KNm/K2
  - per (b, tchunk): 8 fp32r matmuls psum[128, 2048] -> full squared
    distances for 32 kp-groups x 64 shapelets
  - min over shapelets split Vector/GpSimd, DMA out
"""

import sys

for _p in ("/opt/trn_rl_repo",):
    if _p not in sys.path:
        sys.path.insert(0, _p)

import numpy as np

B, T, C = 16, 512, 8
S, KSZ = 64, 32
TOUT = T - KSZ + 1  # 481
NCORES = 8
BPC = B // NCORES  # batches per core
NSIG = BPC * C  # signals per core
EPS = 1e-8
XPAD = 544  # padded signal length (hankel reads up to 511+31)

_cache = {}


def _rap(base, dims, extra=0):
    """Raw AP at base slice's offset (+extra elems) with [step, count] dims."""
    import concourse.bass as bass

    return bass.AP(tensor=base.tensor, offset=base.offset + extra,
                   ap=[list(d) for d in dims])


def _build_nc():
    import concourse.bass as bass
    import concourse.bacc as bacc
    import concourse.tile as tile
    from concourse import mybir
    from contextlib import ExitStack

    f32 = mybir.dt.float32
    f32r = mybir.dt.float32r
    bf16 = mybir.dt.bfloat16
    AX = mybir.AxisListType
    OP = mybir.AluOpType
    ACT = mybir.ActivationFunctionType

    nc = bacc.Bacc("TRN2", target_bir_lowering=False, debug=False)
    x_d = nc.dram_tensor("x", [BPC, T, C], f32, kind="ExternalInput").ap()
    k_d = nc.dram_tensor("kernel", [S, KSZ, C], f32, kind="ExternalInput").ap()
    o_d = nc.dram_tensor("out", [BPC, TOUT, KSZ], f32, kind="ExternalOutput").ap()

    with tile.TileContext(nc) as tc, ExitStack() as ctx:
        const = ctx.enter_context(tc.tile_pool(name="const", bufs=1))
        outp = ctx.enter_context(tc.tile_pool(name="outp", bufs=4))
        dram = ctx.enter_context(tc.tile_pool(name="dram", bufs=1, space="DRAM"))

        Hall = const.tile([65, NSIG * T], f32, tag="Hall")
        Fall = const.tile([65, C * 4 * S], f32, tag="Fall")
        ones8k = const.tile([16, 512], f32, tag="ones8k")
        Xsig = const.tile([C, BPC * T], f32, tag="Xsig")
        Xn = const.tile([C, BPC * XPAD], f32, tag="Xn")

        with tc.tile_pool(name="ldp", bufs=1) as ldp, \
             tc.tile_pool(name="pprep", bufs=1, space="PSUM") as pprep, \
             tc.tile_pool(name="pxp", bufs=2, space="PSUM") as pxp:
            identS = ldp.tile([128, 128], f32, tag="ident")
            from concourse.masks import make_identity
            make_identity(nc, identS[:])

            # ---- zero/one fills (no deps; overlap everything) ----
            nc.gpsimd.memset(Fall[0:64, :], 0.0)
            nc.gpsimd.memset(ones8k[:], 1.0)
            nc.vector.memset(Xn[:], 0.0)

            # ---- x load (contiguous) + PE transpose + strided copies ----
            # X0[p, q] = x_flat[b, p*32 + q]  (t = 4p + q//8, c = q%8)
            for b in range(BPC):
                X0 = ldp.tile([128, 32], f32, tag=f"X0_{b}", name=f"X0_{b}")
                nc.sync.dma_start(
                    out=X0[:],
                    in_=_rap(x_d[b, 0:1, 0:1], [[32, 128], [1, 32]]))
                # per-ts transpose so each PSUM read starts at partition 0
                for ts in range(4):
                    PX = pxp.tile([8, 128], f32, tag="PX")
                    nc.tensor.transpose(
                        PX[:], X0[:, 8 * ts:8 * ts + 8], identS[:])
                    # PX[c, p] = x[b, 4p + ts, c] -> Xsig[c, b*T + 4p + ts]
                    nc.vector.tensor_copy(
                        out=_rap(Xsig[0:1, b * T + ts:b * T + ts + 1],
                                 [[BPC * T, C], [4, 128]]),
                        in_=PX[:])

            # ---- kernel prep chain ----
            KN = ldp.tile([S, KSZ * C], f32, tag="KN")
            nc.gpsimd.dma_start(out=KN[:], in_=k_d.rearrange("s k c -> s (k c)"))
            kst = ldp.tile([S, nc.vector.BN_STATS_DIM], f32, tag="kst")
            nc.vector.bn_stats(out=kst[:], in_=KN[:])
            mvk = ldp.tile([S, nc.vector.BN_AGGR_DIM], f32, tag="mvk")
            nc.vector.bn_aggr(out=mvk[:], in_=kst[:])
            kstd = ldp.tile([S, 1], f32, tag="kstd")
            nc.scalar.activation(out=kstd[:], in_=mvk[:, 1:2], func=ACT.Sqrt)
            nc.vector.tensor_scalar_add(kstd[:], kstd[:], EPS)
            krstd = ldp.tile([S, 1], f32, tag="krstd")
            nc.vector.reciprocal(out=krstd[:], in_=kstd[:])
            kscale = ldp.tile([S, 1], f32, tag="kscale")
            nc.vector.tensor_scalar_mul(kscale[:], krstd[:], -2.0)
            kbias = ldp.tile([S, 1], f32, tag="kbias")
            nc.vector.scalar_tensor_tensor(
                out=kbias[:], in0=mvk[:, 0:1], scalar=2.0, in1=krstd[:],
                op0=OP.mult, op1=OP.mult)
            # KNm = -2 * (KN - mean) * rstd
            KNm = ldp.tile([S, KSZ * C], f32, tag="KNm")
            nc.vector.tensor_scalar(
                out=KNm[:], in0=KN[:], scalar1=kscale[:], scalar2=kbias[:],
                op0=OP.mult, op1=OP.add)
            # K2[kp, s] = sum_c kern_n^2 = 0.25 * sum_c KNm^2
            KN2 = ldp.tile([S, KSZ * C], f32, tag="KN2")
            nc.scalar.activation(out=KN2[:], in_=KNm[:], func=ACT.Square)
            # K2sn columns in (j, ch) order so the F scatter reads
            # consecutive partitions after the transpose
            K2sn = ldp.tile([S, KSZ], f32, tag="K2sn")
            nc.vector.tensor_reduce(
                out=K2sn[:],
                in_=KN2[:].rearrange("s (ch j c) -> s j ch c", ch=C, j=4, c=C),
                axis=AX.X, op=OP.add)
            nc.vector.tensor_scalar_mul(K2sn[:], K2sn[:], 0.25)

            # ---- F staging via PE transposes ----
            # Fall free layout: col = jo*512 + ch*64 + s  (jo outermost)
            # taps transposed: TP[c, jo*512 + ch*64 + s] = KNm[s, (4ch+jo)*8+c]
            TP = pprep.tile([C, 4 * C * S], f32, tag="TP")
            for kp in range(KSZ):
                ch, jo = kp // 4, kp % 4
                nc.tensor.transpose(
                    TP[:, jo * 512 + ch * S:jo * 512 + ch * S + S],
                    KNm[:, kp * C:(kp + 1) * C],
                    identS[0:S, 0:S])
            K2T = pprep.tile([KSZ, S], f32, tag="K2T")
            nc.tensor.transpose(K2T[:], K2sn[:], identS[0:S, 0:S])
            Fx = ldp.tile([C, 4 * C * S], f32, tag="Fx")
            nc.scalar.copy(out=Fx[:].bitcast(f32r), in_=TP[:])
            K2sb = ldp.tile([KSZ, S], f32, tag="K2sb")
            nc.scalar.copy(out=K2sb[:].bitcast(f32r), in_=K2T[:])

            # ---- F scatter: all-2D SBUF->SBUF block DMAs ----
            for j in range(4):
                # taps: rows 8j..8j+8, cols j*512..(j+1)*512
                nc.gpsimd.dma_start(
                    out=Fall[8 * j:8 * j + 8, 512 * j:512 * (j + 1)].bitcast(f32r),
                    in_=Fx[:, 512 * j:512 * (j + 1)].bitcast(f32r))
                # ones blocks for the x^2 window sum
                nc.scalar.dma_start(
                    out=Fall[32 + 8 * j:40 + 8 * j, 512 * j:512 * (j + 1)].bitcast(f32r),
                    in_=ones8k[0:C, :].bitcast(f32r))
                # K2 row segment: Fall[64, j*512 + ch*64 + s] = K2sb[j*8+ch, s]
                nc.gpsimd.dma_start(
                    out=Fall[64:65, 512 * j:512 * (j + 1)].bitcast(f32r),
                    in_=K2sb[C * j:C * (j + 1), :].bitcast(f32r))
            # H ones row
            nc.scalar.dma_start(out=Hall[64:65, :].bitcast(f32r), in_=ones8k[:].bitcast(f32r))

            # ---- x normalize (per batch: stats over its T window) ----
            for b in range(BPC):
                xst = ldp.tile([C, nc.vector.BN_STATS_DIM], f32,
                               tag=f"xst{b}", name=f"xst{b}")
                nc.vector.bn_stats(out=xst[:], in_=Xsig[:, b * T:(b + 1) * T])
                mvx = ldp.tile([C, nc.vector.BN_AGGR_DIM], f32,
                               tag=f"mvx{b}", name=f"mvx{b}")
                nc.vector.bn_aggr(out=mvx[:], in_=xst[:])
                xstd = ldp.tile([C, 1], f32, tag=f"xstd{b}", name=f"xstd{b}")
                nc.scalar.activation(out=xstd[:], in_=mvx[:, 1:2], func=ACT.Sqrt)
                nc.vector.tensor_scalar_add(xstd[:], xstd[:], EPS)
                xrstd = ldp.tile([C, 1], f32, tag=f"xrstd{b}", name=f"xrstd{b}")
                nc.vector.reciprocal(out=xrstd[:], in_=xstd[:])
                xbias = ldp.tile([C, 1], f32, tag=f"xbias{b}", name=f"xbias{b}")
                nc.vector.scalar_tensor_tensor(
                    out=xbias[:], in0=mvx[:, 0:1], scalar=-1.0, in1=xrstd[:],
                    op0=OP.mult, op1=OP.mult)
                nc.vector.tensor_scalar(
                    out=Xn[:, b * XPAD:b * XPAD + T].bitcast(f32r),
                    in0=Xsig[:, b * T:(b + 1) * T], scalar1=xrstd[:],
                    scalar2=xbias[:], op0=OP.mult, op1=OP.add)
            X2n = const.tile([C, BPC * XPAD], f32, tag="X2n")
            nc.scalar.activation(out=X2n[:].bitcast(f32r), in_=Xn[:],
                                 func=ACT.Square)
            XnD = dram.tile([C, BPC * XPAD], f32, tag="XnD")
            nc.sync.dma_start(out=XnD[:].bitcast(f32r), in_=Xn[:].bitcast(f32r))
            X2nD = dram.tile([C, BPC * XPAD], f32, tag="X2nD")
            nc.scalar.dma_start(out=X2nD[:].bitcast(f32r), in_=X2n[:].bitcast(f32r))

            # ---- Hankel x rows via flat DRAM reads; squares in SBUF ----
            dma_engines = [nc.sync, nc.scalar, nc.gpsimd]
            for sig in range(NSIG):
                b, ch = sig // C, sig % C
                off = ch * (BPC * XPAD) + b * XPAD
                eng = dma_engines[sig % 3]
                eng.dma_start(
                    out=Hall[0:KSZ, sig * T:(sig + 1) * T].bitcast(f32r),
                    in_=_rap(XnD[0:1, 0:1],
                             [[1, KSZ], [1, T]], extra=off).bitcast(f32r))
                eng2 = dma_engines[(sig + 1) % 3]
                eng2.dma_start(
                    out=Hall[KSZ:2 * KSZ, sig * T:(sig + 1) * T].bitcast(f32r),
                    in_=_rap(X2nD[0:1, 0:1],
                             [[1, KSZ], [1, T]], extra=off).bitcast(f32r))

        # ---- main: fp32r matmuls + split min-reduce + store ----
        with tc.tile_pool(name="pmm", bufs=2, space="PSUM") as pmm:
            for b in range(BPC):
                for cc in range(4):
                    c0 = cc * 128
                    cnt = 128 if cc < 3 else TOUT - 3 * 128
                    acc = pmm.tile([128, C * 4 * S], f32, tag="acc")
                    for ch in range(C):
                        sig = b * C + ch
                        # rhs strided over Fall's (jo, ch, s) layout: this
                        # ch's 4 jo-blocks of 64 shapelet columns
                        rhs = _rap(Fall[0:65, ch * S:ch * S + 1],
                                   [[2048, 65], [512, 4], [1, S]])
                        nc.tensor.matmul(
                            acc[:, ch * 256:(ch + 1) * 256],
                            lhsT=Hall[:, sig * T + c0:sig * T + c0 + 128].bitcast(f32r),
                            rhs=rhs.bitcast(f32r),
                            start=True, stop=True)
                    PM = outp.tile([128, KSZ], f32, tag="PM")
                    nc.vector.tensor_reduce(
                        out=PM[:],
                        in_=acc[:].rearrange("p (g s) -> p g s", s=S),
                        axis=AX.X, op=OP.min)
                    nc.sync.dma_start(
                        out=o_d[b, c0:c0 + cnt, :], in_=PM[0:cnt, :])

    nc.compile()
    return nc


def get_nc():
    if "nc" not in _cache:
        _cache["nc"] = _build_nc()
    return _cache["nc"]


def kernel(x: np.ndarray, kernel: np.ndarray) -> np.ndarray:
    from concourse.bass_utils import run_bass_kernel_spmd

    nc = get_nc()
    x = np.ascontiguousarray(x, dtype=np.float32)
    kern = np.ascontiguousarray(kernel, dtype=np.float32)
    in_maps = [
        {"x": x[i * BPC:(i + 1) * BPC], "kernel": kern} for i in range(NCORES)
    ]
    res = run_bass_kernel_spmd(nc, in_maps, core_ids=list(range(NCORES)))
    return np.concatenate([r["out"] for r in res.results], axis=0)


if __name__ == "__main__":
    rng = np.random.default_rng(0)
    x = rng.standard_normal((B, T, C), dtype=np.float32)
    k = rng.uniform(-0.05, 0.05, (S, KSZ, C)).astype(np.float32)
    out = kernel(x=x, kernel=k)
    print(out.shape, out.dtype)
